# revision 12
# baseline (speedup 1.0000x reference)
"""Trainium2 Bass kernel for the e3nn-style point kernel:

    out[z, i, j] = sum_{y,w} Q[i,j,y,w] * Ysh[z,y] * Rad[z,w]      (+ K0 fallback
                                                                     for |r|==0)
    Ysh = real spherical harmonics l=0,1,2 of d = r/|r|  (component norm)
    Rad = relu(|r| * W1 + b1) @ W2 + b2

Data-parallel over N across 8 cores, NP=8192 points each, z = 64*p + t
(partition-major) so every HBM transfer is contiguous.

v2 design -- features are built DIRECTLY feature-major (the old kernel's
14MB point-major->feature-major DMA-xbar transpose dominated its runtime):
  - radii + Ysh computed point-major (cheap DVE ops), packed fp16 into
    [128, 10, 128] (slot 0 = radii, 1..9 = Ysh), ONE xbar transpose ->
    staged (s, t, p)-major in DRAM.
  - Per chunk of 512 points: radii row [1,512] (one upfront bulk load)
    feeds the K=1 W1 outer-product matmul -> h^T [128h, 512z]; relu on
    ACT (per-partition bias b1); ONE K=128 matmul lhsT=W2 -> RadT
    [96w, 512z] PSUM; ACT copy + per-partition b2 bias -> fp16 SBUF.
  - Feature K-tiles, f = 96*y + w: tile 0 = y0 rows = RadT itself (used
    directly as the k=0 moving operand, zero build cost); tiles 1..6 =
    the remaining 768 rows (exactly 6x128), built by 6 paired DVE/Pool
    multiplies: ft = RadT-partition-view * B_y, where B_y [96, 512] is
    Ysh row y partition-broadcast, loaded by ONE stride-0 DMA per chunk.
    (The segment pattern repeats every 3 tiles since 3*128 = 4*96, so
    tiles t and t+3 pair into single two-free-dim instructions.)
  - Main GEMM is ij-major (moving = features, N=512 z-columns; stationary
    = Qstack ij-halves) so PE runs at streaming rate instead of being
    SEQ-dispatch-bound on N=256 matmuls: out^T[ij, z] accumulated over 7
    K-tiles -> fp16 -> DRAM [2, 128, NP]; the host unscrambles (the
    harness' `unscramble` hook) back to [NP, 256].
"""

import math
from contextlib import ExitStack

import numpy as np

import concourse.bass as bass
import concourse.mybir as mybir
import concourse.tile as tile
from concourse import bacc
from concourse._compat import with_exitstack
from concourse.bass import ds, ts
from concourse.bass_utils import run_bass_kernel_spmd

F32 = mybir.dt.float32
F16 = mybir.dt.float16
AF = mybir.ActivationFunctionType
OP = mybir.AluOpType

N_TOTAL = 65536
N_CORES = 8
NP = N_TOTAL // N_CORES          # 8192 points per core
NT = NP // 128                   # 64 z-tiles of 128 points (z = 64*p + t)
NCH = NT // 4                    # 16 chunks of 4 z-tiles (512 points)
H = 128                          # MLP hidden
W = 96                           # MLP out / radial channels
NY = 9                           # spherical harmonics
IJ = 256                         # 16*16 outputs
KT = 7                           # K-tiles: t0 = y0 (K=96), t1..t6 = 128 rows

SQ3 = math.sqrt(3.0)
SQ5 = math.sqrt(5.0)
SQ15 = math.sqrt(15.0)

# Segments for tiles t=1..3 (tiles t+3 mirror with y+4); (t, a, b, y, w0):
# tile t rows [a, b) hold features (y, w0 + (p - a)).  The w-range assigned
# to each y-block part is chosen so every SBUF access is partition-window
# aligned (hardware: base 32 -> span <= 32, base 64 -> span <= 64): a y
# block split 32+64 across tiles puts w 64:96 in the 32-part and w 0:64 in
# the 64-part.  Q is host-packed to match (see pack_weights).
SEGS3 = [
    (1, 0, 96, 1, 0),      # y1 full           out (0,96)   in (0,96)
    (1, 96, 128, 2, 64),   # y2 part: w 64:96  out (96,32)  in (64,32)
    (2, 0, 64, 2, 0),      # y2 part: w 0:64   out (0,64)   in (0,64)
    (2, 64, 128, 3, 0),    # y3 part: w 0:64   out (64,64)  in (0,64)
    (3, 0, 96, 4, 0),      # y4 full           out (0,96)   in (0,96)
    (3, 96, 128, 3, 64),   # y3 part: w 64:96  out (96,32)  in (64,32)
]
# Pool-engine assignment for the paired multiplies, keyed by (t, a):
POOL_SEGS = {(3, 0), (3, 96)}


@with_exitstack
def _emit(ctx: ExitStack, tc: tile.TileContext, r_ext, q_ext, w1_ext, b1_ext,
          w2_ext, b2_ext, out_ext):
    nc = tc.nc

    consts = ctx.enter_context(tc.tile_pool(name="consts", bufs=1))
    work = ctx.enter_context(tc.tile_pool(name="work", bufs=1))

    # ---------------- constants (r first: it gates the critical path) ------
    r_sb = work.tile([128, NT, 3], F32)
    nc.sync.dma_start(out=r_sb, in_=r_ext.rearrange("(p t) c -> p t c", t=NT))
    w1_sb = consts.tile([1, H], F16)
    nc.sync.dma_start(out=w1_sb, in_=w1_ext[:, :])
    b1_sb = consts.tile([H, 1], F32)
    nc.sync.dma_start(out=b1_sb, in_=b1_ext.rearrange("(h o) -> h o", o=1))
    w2_sb = consts.tile([H, W], F16)
    nc.sync.dma_start(out=w2_sb, in_=w2_ext[:, :])
    b2_sb = consts.tile([W, 1], F32)
    nc.sync.dma_start(out=b2_sb, in_=b2_ext.rearrange("(w o) -> w o", o=1))

    # Qstack: [128, KT, IJ] fp16 (host-prepacked; tile 0 rows 96:128 unused)
    qmat = consts.tile([128, KT, IJ], F16)
    nc.sync.dma_start(out=qmat, in_=q_ext[:, :, :])

    # ---------------- point-major precomputation ----------------
    rsq = work.tile([128, NT, 3], F32)
    nc.vector.tensor_mul(rsq, r_sb, r_sb)
    rad2 = work.tile([128, NT], F32)
    nc.vector.tensor_reduce(rad2, rsq, axis=mybir.AxisListType.X, op=OP.add)
    radii = work.tile([128, NT], F32)
    nc.scalar.activation(radii, rad2, AF.Sqrt)
    invr = work.tile([128, NT], F32)
    nc.vector.reciprocal(invr, rad2)                    # 1/rad^2
    nc.vector.tensor_mul(invr, invr, radii)             # -> 1/rad

    d = work.tile([128, NT, 3], F32)
    for c in range(3):
        nc.vector.tensor_mul(d[:, :, c], r_sb[:, :, c], invr)
    e = work.tile([128, NT, 3], F32)
    nc.vector.tensor_scalar_mul(e, d, SQ15)
    g = work.tile([128, NT, 3], F32)
    nc.vector.tensor_scalar_mul(g, e, 0.5)

    # pk[p, s, t]: s=0 radii, s=1..9 Ysh_{s-1}[z], fp16; slices padded to 128
    # cols (xbar transpose needs free % 128 == 0; only cols 0:NT matter)
    pk = work.tile([128, 10, 128], F16)
    nc.vector.tensor_copy(pk[:, 0, 0:NT], radii)
    nc.vector.memset(pk[:, 1, 0:NT], 1.0)
    nc.vector.tensor_scalar_mul(pk[:, 2, 0:NT], d[:, :, 1], SQ3)
    nc.vector.tensor_scalar_mul(pk[:, 3, 0:NT], d[:, :, 2], SQ3)
    nc.vector.tensor_scalar_mul(pk[:, 4, 0:NT], d[:, :, 0], SQ3)
    nc.vector.tensor_mul(pk[:, 5, 0:NT], e[:, :, 0], d[:, :, 1])
    nc.vector.tensor_mul(pk[:, 6, 0:NT], e[:, :, 1], d[:, :, 2])
    t2 = work.tile([128, NT], F32)
    nc.vector.tensor_mul(t2, d[:, :, 2], d[:, :, 2])
    nc.vector.tensor_scalar(pk[:, 7, 0:NT], t2, 1.5 * SQ5, -0.5 * SQ5,
                            op0=OP.mult, op1=OP.add)
    nc.vector.tensor_mul(pk[:, 8, 0:NT], e[:, :, 0], d[:, :, 2])
    su = work.tile([128, NT], F32)
    sv = work.tile([128, NT], F32)
    nc.vector.tensor_mul(su, g[:, :, 0], d[:, :, 0])
    nc.vector.tensor_mul(sv, g[:, :, 1], d[:, :, 1])
    nc.vector.tensor_sub(pk[:, 9, 0:NT], su, sv)

    # ONE xbar transpose of all 10 slices: pkT[p, s, z] = pk[z, 128s + p];
    # rows 0:NT of each slot hold the transposed slice (t on partitions).
    # Stage (s, t, p)-major in DRAM so chunk rows are contiguous [1, 512].
    pkT = work.tile([128, 10, 128], F16)
    nc.sync.dma_start_transpose(out=pkT, in_=pk)
    stage = nc.dram_tensor("stage_scratch", [10, NT, 128], F16)
    nc.sync.dma_start(out=stage.rearrange("s t p -> t s p"),
                      in_=pkT[0:NT, :, :])
    stage_rows = stage.rearrange("s t p -> s () (t p)")
    stage_b = stage.rearrange("s t p -> () s (t p)")

    # all radii rows in one load: rall[0, 512*ch : 512*(ch+1)] = chunk ch
    rall = work.tile([1, NP], F16)
    nc.sync.dma_start(out=rall, in_=stage_rows[0, :, :])
    # y7/y8 rows: B slots 6/7 are built by PE K=1 matmuls (ones x ysh_row)
    # + ACT copies instead of DMA broadcast, relieving the DMA queues
    yall7 = work.tile([1, NP], F16)
    nc.sync.dma_start(out=yall7, in_=stage_rows[8, :, :])
    yall8 = work.tile([1, NP], F16)
    nc.sync.dma_start(out=yall8, in_=stage_rows[9, :, :])
    ones96 = consts.tile([1, 96], F16)
    nc.vector.memset(ones96, 1.0)

    # ---------------- main loop ----------------
    p_ht = ctx.enter_context(tc.tile_pool(name="ht", bufs=3))
    p_rt = ctx.enter_context(tc.tile_pool(name="rt", bufs=4))
    p_b = ctx.enter_context(tc.tile_pool(name="bb", bufs=3))
    p_ft = ctx.enter_context(tc.tile_pool(name="ft", bufs=3))
    p_ost = ctx.enter_context(tc.tile_pool(name="ost", bufs=3))
    ps_h = ctx.enter_context(tc.tile_pool(name="ps_h", bufs=1, space="PSUM"))
    ps_r = ctx.enter_context(tc.tile_pool(name="ps_r", bufs=2, space="PSUM"))
    ps_o = ctx.enter_context(tc.tile_pool(name="ps_o", bufs=2, space="PSUM"))
    ps_b = ctx.enter_context(tc.tile_pool(name="ps_b", bufs=1, space="PSUM"))

    b_tiles = [None] * (NCH // 2)

    def load_b(cp):
        # ONE DMA per 2 chunks (2KB rows): B[p, i, z2] = Ysh_{i+1}[z-col];
        # alternate the two HWDGE queues (SP / ACT) for bandwidth
        bt = p_b.tile([96, 8, 1024], F16, tag="B", name=f"bt{cp}")
        b_tiles[cp] = bt
        eng = nc.sync if cp % 2 == 0 else nc.scalar
        eng.dma_start(
            out=bt[:, 0:6, :],
            in_=stage_b[:, 2:8, ds(1024 * cp, 1024)].partition_broadcast(96))

    load_b(0)

    # RadT is produced LOOKAHEAD chunks ahead of its consumers so the PE's
    # main-GEMM(n) and the DVE/Pool multiplies(n+1) can overlap instead of
    # ping-ponging (radT(n+1) would otherwise be queued on PE after main(n)).
    LOOKAHEAD = 2
    radts = [None] * NCH

    def make_radt(ch):
        # hidden h^T: hp[h, c] = W1[h] * radii(z(c))
        hp = ps_h.tile([128, 512], F32, tag="hp", name=f"hp{ch}")
        nc.tensor.matmul(out=hp, lhsT=w1_sb, rhs=rall[:, ds(512 * ch, 512)],
                         start=True, stop=True)
        ht = p_ht.tile([128, 512], F16, tag="ht", name=f"ht{ch}")
        nc.scalar.activation(ht, hp, AF.Relu, bias=b1_sb, scale=1.0)
        # RadT[w, z] = sum_h W2[h, w] * ht[h, z]  (+ b2 via ACT bias)
        rp = ps_r.tile([W, 512], F32, tag="rp", name=f"rp{ch}")
        nc.tensor.matmul(out=rp, lhsT=w2_sb, rhs=ht, start=True, stop=True)
        radT = p_rt.tile([W, 512], F16, tag="radT", name=f"radT{ch}")
        nc.scalar.activation(radT, rp, AF.Identity, bias=b2_sb, scale=1.0)
        radts[ch] = radT

    for c in range(LOOKAHEAD):
        make_radt(c)

    for ch in range(NCH):
        if ch % 2 == 0 and ch // 2 + 1 < NCH // 2:
            load_b(ch // 2 + 1)
        if ch + LOOKAHEAD < NCH:
            make_radt(ch + LOOKAHEAD)
        radT = radts[ch]

        # feature tiles 1..6 via paired multiplies (tiles t and t+3 share
        # partition/w pattern; y differs by 4): ft[p, slot, z] with
        # slot = 3*u + (t-1), u in {0,1}; B slot = 4*u + (y-1).
        bt = b_tiles[ch // 2]
        zo = 512 * (ch % 2)
        for yi, yrow in ((6, yall7), (7, yall8)):
            bp = ps_b.tile([96, 512], F32, tag="bp", name=f"bp{ch}_{yi}")
            nc.tensor.matmul(out=bp, lhsT=ones96,
                             rhs=yrow[:, ds(512 * ch, 512)],
                             start=True, stop=True)
            nc.scalar.copy(bt[0:96, yi, ds(zo, 512)], bp)
        ft = p_ft.tile([128, KT - 1, 512], F16, tag="ft")
        ftv = ft.rearrange("p (u v) z -> p v u z", v=3)
        btv = bt.rearrange("p (u v) z -> p v u z", v=4)
        for (t, a, b, y, w0) in SEGS3:
            h = b - a
            eng = nc.gpsimd if (t, a) in POOL_SEGS else nc.vector
            eng.tensor_mul(
                ftv[a:b, t - 1, :, :],
                radT[w0:w0 + h, None, :].broadcast_to([h, 2, 512]),
                btv[w0:w0 + h, y - 1, :, ds(zo, 512)])

        # main GEMM (ij-major): outT[ij_half, z] += Q_half_k^T @ ft_k
        op = ps_o.tile([128, 2, 512], F32, tag="op")
        for half in range(2):
            nc.tensor.matmul(out=op[:, half, :],
                             lhsT=qmat[0:W, 0, ds(128 * half, 128)],
                             rhs=radT, start=True, stop=False)
            for k in range(1, KT):
                nc.tensor.matmul(out=op[:, half, :],
                                 lhsT=qmat[:, k, ds(128 * half, 128)],
                                 rhs=ft[:, k - 1, :],
                                 start=False, stop=(k == KT - 1))
        # batch output per 2 chunks: 4KB DRAM lines per partition
        if ch % 2 == 0:
            ost = p_ost.tile([128, 2, 2, 512], F16, tag="ost",
                             name=f"ost{ch}")
        nc.scalar.copy(ost[:, ch % 2, :, :], op)
        if ch % 2 == 1:
            nc.sync.dma_start(out=out_ext[:, ch // 2, :, :, :], in_=ost)


def build_nc(repeat: int = 1) -> bass.Bass:
    nc = bacc.Bacc()
    r_ext = nc.declare_dram_parameter("r", [NP, 3], F32, isOutput=False)
    q_ext = nc.declare_dram_parameter("qstack", [128, KT, IJ], F16,
                                      isOutput=False)
    w1_ext = nc.declare_dram_parameter("w1", [1, H], F16, isOutput=False)
    b1_ext = nc.declare_dram_parameter("b1", [H], F32, isOutput=False)
    w2_ext = nc.declare_dram_parameter("w2", [H, W], F16, isOutput=False)
    b2_ext = nc.declare_dram_parameter("b2", [W], F32, isOutput=False)
    out_ext = nc.declare_dram_parameter("out", [128, NCH // 2, 2, 2, 512],
                                        F16, isOutput=True)
    with tile.TileContext(nc) as tc:
        if repeat == 1:
            _emit(tc, r_ext, q_ext, w1_ext, b1_ext, w2_ext, b2_ext, out_ext)
        else:
            with tc.For_i(0, repeat, 1, staggered_reset=True):
                _emit(tc, r_ext, q_ext, w1_ext, b1_ext, w2_ext, b2_ext,
                      out_ext)
    nc.compile()
    return nc


def pack_weights(Q):
    """qstack tile 0 = y0 rows (w 0:96); tiles 1..6 rows follow SEGS3 (and
    its +3/+4 mirror): row p of tile t = Q[(y, w0 + p - a)].  Laid out
    [128, KT, IJ] fp16; tile 0 rows 96:128 zero."""
    Q = np.asarray(Q, np.float32)
    qmat = Q.transpose(2, 3, 0, 1).reshape(NY * W, IJ)      # [(y,w), (i,j)]
    qstack = np.zeros((KT, 128, IJ), np.float16)
    qstack[0, 0:W] = qmat[0:W].astype(np.float16)
    for (t, a, b, y, w0) in SEGS3:
        for u in range(2):
            rows = qmat[96 * (y + 4 * u) + w0:96 * (y + 4 * u) + w0 + b - a]
            qstack[t + 3 * u, a:b] = rows.astype(np.float16)
    return np.ascontiguousarray(qstack.transpose(1, 0, 2))


def make_in_map(inputs, core):
    r = np.ascontiguousarray(np.asarray(inputs["r"], np.float32))
    return dict(
        r=r[core * NP:(core + 1) * NP],
        qstack=pack_weights(inputs["Q"]),
        w1=np.ascontiguousarray(
            np.asarray(inputs["W1"], np.float32).astype(np.float16)),
        b1=np.ascontiguousarray(np.asarray(inputs["b1"], np.float32)),
        w2=np.ascontiguousarray(
            np.asarray(inputs["W2"], np.float32).astype(np.float16)),
        b2=np.ascontiguousarray(np.asarray(inputs["b2"], np.float32)),
    )


# device element a[i, cp, cpar, h, 128*j + p] = out[z, 128*h + i] with
# z = 64*p + 4*(2*cp + cpar) + j
_C = np.arange(NP)  # flat (cp, cpar, j, p) index = 1024*cp + 512*cpar + c
_ZMAP = (64 * (_C % 128) + 4 * (_C // 512) + (_C // 128) % 4)


def unscramble(a):
    """Device out [128, NCH//2, 2, 2, 512] -> [NP, IJ]."""
    a = np.asarray(a).reshape(128, NCH // 2, 2, 2, 512)
    # -> [h, i, cp, cpar, z2] -> [IJ, NP]
    a2 = a.transpose(3, 0, 1, 2, 4).reshape(IJ, NP)
    out = np.empty((NP, IJ), dtype=a2.dtype)
    out[_ZMAP] = a2.T
    return out


_NC_CACHE = None


def _get_nc():
    global _NC_CACHE
    if _NC_CACHE is None:
        _NC_CACHE = build_nc()
    return _NC_CACHE


def kernel(r, Q, W1, b1, W2, b2, K0):
    r = np.ascontiguousarray(np.asarray(r, dtype=np.float32))
    inputs = dict(r=r, Q=Q, W1=W1, b1=b1, W2=W2, b2=b2)
    in_maps = [make_in_map(inputs, i) for i in range(N_CORES)]
    res = run_bass_kernel_spmd(_get_nc(), in_maps, list(range(N_CORES)))
    out = np.concatenate([unscramble(res.results[i]["out"])
                          for i in range(N_CORES)], 0)
    out = out.reshape(N_TOTAL, 16, 16).astype(np.float32)
    # exact reference semantics for |r| == 0 points (K0 fallback)
    zero = ~(np.linalg.norm(r, axis=1) > 0.0)
    if zero.any():
        out[zero] = np.asarray(K0, np.float32)[None]
    return out


# revision 13
# speedup vs baseline: 1.0772x; 1.0772x over previous
"""Trainium2 Bass kernel for the e3nn-style point kernel:

    out[z, i, j] = sum_{y,w} Q[i,j,y,w] * Ysh[z,y] * Rad[z,w]      (+ K0 fallback
                                                                     for |r|==0)
    Ysh = real spherical harmonics l=0,1,2 of d = r/|r|  (component norm)
    Rad = relu(|r| * W1 + b1) @ W2 + b2

Data-parallel over N across 8 cores, NP=8192 points each, z = 64*p + t
(partition-major) so every HBM transfer is contiguous.

v2 design -- features are built DIRECTLY feature-major (the old kernel's
14MB point-major->feature-major DMA-xbar transpose dominated its runtime):
  - radii + Ysh computed point-major (cheap DVE ops), packed fp16 into
    [128, 10, 128] (slot 0 = radii, 1..9 = Ysh), ONE xbar transpose ->
    staged (s, t, p)-major in DRAM.
  - Per chunk of 512 points: radii row [1,512] (one upfront bulk load)
    feeds the K=1 W1 outer-product matmul -> h^T [128h, 512z]; relu on
    ACT (per-partition bias b1); ONE K=128 matmul lhsT=W2 -> RadT
    [96w, 512z] PSUM; ACT copy + per-partition b2 bias -> fp16 SBUF.
  - Feature K-tiles, f = 96*y + w: tile 0 = y0 rows = RadT itself (used
    directly as the k=0 moving operand, zero build cost); tiles 1..6 =
    the remaining 768 rows (exactly 6x128), built by 6 paired DVE/Pool
    multiplies: ft = RadT-partition-view * B_y, where B_y [96, 512] is
    Ysh row y partition-broadcast, loaded by ONE stride-0 DMA per chunk.
    (The segment pattern repeats every 3 tiles since 3*128 = 4*96, so
    tiles t and t+3 pair into single two-free-dim instructions.)
  - Main GEMM is ij-major (moving = features, N=512 z-columns; stationary
    = Qstack ij-halves) so PE runs at streaming rate instead of being
    SEQ-dispatch-bound on N=256 matmuls: out^T[ij, z] accumulated over 7
    K-tiles -> fp16 -> DRAM [2, 128, NP]; the host unscrambles (the
    harness' `unscramble` hook) back to [NP, 256].
"""

import math
from contextlib import ExitStack

import numpy as np

import concourse.bass as bass
import concourse.mybir as mybir
import concourse.tile as tile
from concourse import bacc
from concourse._compat import with_exitstack
from concourse.bass import ds, ts
from concourse.bass_utils import run_bass_kernel_spmd

F32 = mybir.dt.float32
F16 = mybir.dt.float16
AF = mybir.ActivationFunctionType
OP = mybir.AluOpType

N_TOTAL = 65536
N_CORES = 8
NP = N_TOTAL // N_CORES          # 8192 points per core
NT = NP // 128                   # 64 z-tiles of 128 points (z = 64*p + t)
NCH = NT // 4                    # 16 chunks of 4 z-tiles (512 points)
H = 128                          # MLP hidden
W = 96                           # MLP out / radial channels
NY = 9                           # spherical harmonics
IJ = 256                         # 16*16 outputs
KT = 7                           # K-tiles: t0 = y0 (K=96), t1..t6 = 128 rows

SQ3 = math.sqrt(3.0)
SQ5 = math.sqrt(5.0)
SQ15 = math.sqrt(15.0)

# Segments for tiles t=1..3 (tiles t+3 mirror with y+4); (t, a, b, y, w0):
# tile t rows [a, b) hold features (y, w0 + (p - a)).  The w-range assigned
# to each y-block part is chosen so every SBUF access is partition-window
# aligned (hardware: base 32 -> span <= 32, base 64 -> span <= 64): a y
# block split 32+64 across tiles puts w 64:96 in the 32-part and w 0:64 in
# the 64-part.  Q is host-packed to match (see pack_weights).
SEGS3 = [
    (1, 0, 96, 1, 0),      # y1 full           out (0,96)   in (0,96)
    (1, 96, 128, 2, 64),   # y2 part: w 64:96  out (96,32)  in (64,32)
    (2, 0, 64, 2, 0),      # y2 part: w 0:64   out (0,64)   in (0,64)
    (2, 64, 128, 3, 0),    # y3 part: w 0:64   out (64,64)  in (0,64)
    (3, 0, 96, 4, 0),      # y4 full           out (0,96)   in (0,96)
    (3, 96, 128, 3, 64),   # y3 part: w 64:96  out (96,32)  in (64,32)
]
# Pool-engine assignment for the paired multiplies, keyed by (t, a):
POOL_SEGS = {(3, 0), (3, 96)}


@with_exitstack
def _emit(ctx: ExitStack, tc: tile.TileContext, r_ext, q_ext, w1_ext, b1_ext,
          w2_ext, b2_ext, out_ext):
    nc = tc.nc

    consts = ctx.enter_context(tc.tile_pool(name="consts", bufs=1))
    work = ctx.enter_context(tc.tile_pool(name="work", bufs=1))

    # ---------------- constants (r first: it gates the critical path) ------
    r_sb = work.tile([128, NT, 3], F32)
    nc.sync.dma_start(out=r_sb, in_=r_ext.rearrange("(p t) c -> p t c", t=NT))
    w1_sb = consts.tile([1, H], F16)
    nc.sync.dma_start(out=w1_sb, in_=w1_ext[:, :])
    b1_sb = consts.tile([H, 1], F32)
    nc.sync.dma_start(out=b1_sb, in_=b1_ext.rearrange("(h o) -> h o", o=1))
    w2_sb = consts.tile([H, W], F16)
    nc.sync.dma_start(out=w2_sb, in_=w2_ext[:, :])
    b2_sb = consts.tile([W, 1], F32)
    nc.sync.dma_start(out=b2_sb, in_=b2_ext.rearrange("(w o) -> w o", o=1))

    # Qstack: [128, KT, IJ] fp16 (host-prepacked; tile 0 rows 96:128 unused)
    qmat = consts.tile([128, KT, IJ], F16)
    nc.sync.dma_start(out=qmat, in_=q_ext[:, :, :])

    # ---------------- point-major precomputation ----------------
    rsq = work.tile([128, NT, 3], F32)
    nc.vector.tensor_mul(rsq, r_sb, r_sb)
    rad2 = work.tile([128, NT], F32)
    nc.vector.tensor_reduce(rad2, rsq, axis=mybir.AxisListType.X, op=OP.add)
    radii = work.tile([128, NT], F32)
    nc.scalar.activation(radii, rad2, AF.Sqrt)
    invr = work.tile([128, NT], F32)
    nc.vector.reciprocal(invr, rad2)                    # 1/rad^2
    nc.vector.tensor_mul(invr, invr, radii)             # -> 1/rad

    d = work.tile([128, NT, 3], F32)
    for c in range(3):
        nc.vector.tensor_mul(d[:, :, c], r_sb[:, :, c], invr)
    e = work.tile([128, NT, 3], F32)
    nc.vector.tensor_scalar_mul(e, d, SQ15)
    g = work.tile([128, NT, 3], F32)
    nc.vector.tensor_scalar_mul(g, e, 0.5)

    # pk[p, s, t]: s=0 radii, s=1..9 Ysh_{s-1}[z], fp16; slices padded to 128
    # cols (xbar transpose needs free % 128 == 0; only cols 0:NT matter)
    pk = work.tile([128, 10, 128], F16)
    nc.vector.tensor_copy(pk[:, 0, 0:NT], radii)
    nc.vector.memset(pk[:, 1, 0:NT], 1.0)
    nc.vector.tensor_scalar_mul(pk[:, 2, 0:NT], d[:, :, 1], SQ3)
    nc.vector.tensor_scalar_mul(pk[:, 3, 0:NT], d[:, :, 2], SQ3)
    nc.vector.tensor_scalar_mul(pk[:, 4, 0:NT], d[:, :, 0], SQ3)
    nc.vector.tensor_mul(pk[:, 5, 0:NT], e[:, :, 0], d[:, :, 1])
    nc.vector.tensor_mul(pk[:, 6, 0:NT], e[:, :, 1], d[:, :, 2])
    t2 = work.tile([128, NT], F32)
    nc.vector.tensor_mul(t2, d[:, :, 2], d[:, :, 2])
    nc.vector.tensor_scalar(pk[:, 7, 0:NT], t2, 1.5 * SQ5, -0.5 * SQ5,
                            op0=OP.mult, op1=OP.add)
    nc.vector.tensor_mul(pk[:, 8, 0:NT], e[:, :, 0], d[:, :, 2])
    su = work.tile([128, NT], F32)
    sv = work.tile([128, NT], F32)
    nc.vector.tensor_mul(su, g[:, :, 0], d[:, :, 0])
    nc.vector.tensor_mul(sv, g[:, :, 1], d[:, :, 1])
    nc.vector.tensor_sub(pk[:, 9, 0:NT], su, sv)

    # ONE xbar transpose of all 10 slices: pkT[p, s, z] = pk[z, 128s + p];
    # rows 0:NT of each slot hold the transposed slice (t on partitions).
    # Stage (s, t, p)-major in DRAM so chunk rows are contiguous [1, 512].
    pkT = work.tile([128, 10, 128], F16)
    nc.sync.dma_start_transpose(out=pkT, in_=pk)
    stage = nc.dram_tensor("stage_scratch", [10, NT, 128], F16)
    nc.sync.dma_start(out=stage.rearrange("s t p -> t s p"),
                      in_=pkT[0:NT, :, :])
    stage_rows = stage.rearrange("s t p -> s () (t p)")
    stage_b = stage.rearrange("s t p -> () s (t p)")

    # all radii rows in one load: rall[0, 512*ch : 512*(ch+1)] = chunk ch
    rall = work.tile([1, NP], F16)
    nc.sync.dma_start(out=rall, in_=stage_rows[0, :, :])

    # ---------------- main loop ----------------
    p_ht = ctx.enter_context(tc.tile_pool(name="ht", bufs=3))
    p_rt = ctx.enter_context(tc.tile_pool(name="rt", bufs=4))
    p_b = ctx.enter_context(tc.tile_pool(name="bb", bufs=3))
    p_ft = ctx.enter_context(tc.tile_pool(name="ft", bufs=3))
    p_ost = ctx.enter_context(tc.tile_pool(name="ost", bufs=3))
    ps_h = ctx.enter_context(tc.tile_pool(name="ps_h", bufs=2, space="PSUM"))
    ps_r = ctx.enter_context(tc.tile_pool(name="ps_r", bufs=2, space="PSUM"))
    ps_o = ctx.enter_context(tc.tile_pool(name="ps_o", bufs=2, space="PSUM"))

    b_tiles = [None] * (NCH // 2)

    def load_b(cp):
        # ONE DMA per 2 chunks (2KB rows): B[p, i, z2] = Ysh_{i+1}[z-col];
        # alternate the two HWDGE queues (SP / ACT) for bandwidth
        bt = p_b.tile([96, 8, 1024], F16, tag="B", name=f"bt{cp}")
        b_tiles[cp] = bt
        eng = nc.sync if cp % 2 == 0 else nc.scalar
        eng.dma_start(
            out=bt,
            in_=stage_b[:, 2:10, ds(1024 * cp, 1024)].partition_broadcast(96))

    load_b(0)

    # RadT is produced LOOKAHEAD chunks ahead of its consumers so the PE's
    # main-GEMM(n) and the DVE/Pool multiplies(n+1) can overlap instead of
    # ping-ponging (radT(n+1) would otherwise be queued on PE after main(n)).
    LOOKAHEAD = 2
    radts = [None] * NCH

    def make_radt(ch):
        # hidden h^T: hp[h, c] = W1[h] * radii(z(c))
        hp = ps_h.tile([128, 512], F32, tag="hp", name=f"hp{ch}")
        nc.tensor.matmul(out=hp, lhsT=w1_sb, rhs=rall[:, ds(512 * ch, 512)],
                         start=True, stop=True)
        ht = p_ht.tile([128, 512], F16, tag="ht", name=f"ht{ch}")
        nc.scalar.activation(ht, hp, AF.Relu, bias=b1_sb, scale=1.0)
        # RadT[w, z] = sum_h W2[h, w] * ht[h, z]  (+ b2 via ACT bias)
        rp = ps_r.tile([W, 512], F32, tag="rp", name=f"rp{ch}")
        nc.tensor.matmul(out=rp, lhsT=w2_sb, rhs=ht, start=True, stop=True)
        radT = p_rt.tile([W, 512], F16, tag="radT", name=f"radT{ch}")
        nc.scalar.activation(radT, rp, AF.Identity, bias=b2_sb, scale=1.0)
        radts[ch] = radT

    for c in range(LOOKAHEAD):
        make_radt(c)

    for ch in range(NCH):
        if ch % 2 == 0 and ch // 2 + 1 < NCH // 2:
            load_b(ch // 2 + 1)
        if ch + LOOKAHEAD < NCH:
            make_radt(ch + LOOKAHEAD)
        radT = radts[ch]

        # feature tiles 1..6 via paired multiplies (tiles t and t+3 share
        # partition/w pattern; y differs by 4): ft[p, slot, z] with
        # slot = 3*u + (t-1), u in {0,1}; B slot = 4*u + (y-1).
        bt = b_tiles[ch // 2]
        zo = 512 * (ch % 2)
        ft = p_ft.tile([128, KT - 1, 512], F16, tag="ft")
        ftv = ft.rearrange("p (u v) z -> p v u z", v=3)
        btv = bt.rearrange("p (u v) z -> p v u z", v=4)
        for (t, a, b, y, w0) in SEGS3:
            h = b - a
            eng = nc.gpsimd if (t, a) in POOL_SEGS else nc.vector
            eng.tensor_mul(
                ftv[a:b, t - 1, :, :],
                radT[w0:w0 + h, None, :].broadcast_to([h, 2, 512]),
                btv[w0:w0 + h, y - 1, :, ds(zo, 512)])

        # main GEMM (ij-major): outT[ij_half, z] += Q_half_k^T @ ft_k
        op = ps_o.tile([128, 2, 512], F32, tag="op")
        for half in range(2):
            nc.tensor.matmul(out=op[:, half, :],
                             lhsT=qmat[0:W, 0, ds(128 * half, 128)],
                             rhs=radT, start=True, stop=False)
            for k in range(1, KT):
                nc.tensor.matmul(out=op[:, half, :],
                                 lhsT=qmat[:, k, ds(128 * half, 128)],
                                 rhs=ft[:, k - 1, :],
                                 start=False, stop=(k == KT - 1))
        # batch output per 2 chunks: 4KB DRAM lines per partition
        if ch % 2 == 0:
            ost = p_ost.tile([128, 2, 2, 512], F16, tag="ost",
                             name=f"ost{ch}")
        nc.scalar.copy(ost[:, ch % 2, :, :], op)
        if ch % 2 == 1:
            nc.sync.dma_start(out=out_ext[:, ch // 2, :, :, :], in_=ost)


def build_nc(repeat: int = 1) -> bass.Bass:
    nc = bacc.Bacc()
    r_ext = nc.declare_dram_parameter("r", [NP, 3], F32, isOutput=False)
    q_ext = nc.declare_dram_parameter("qstack", [128, KT, IJ], F16,
                                      isOutput=False)
    w1_ext = nc.declare_dram_parameter("w1", [1, H], F16, isOutput=False)
    b1_ext = nc.declare_dram_parameter("b1", [H], F32, isOutput=False)
    w2_ext = nc.declare_dram_parameter("w2", [H, W], F16, isOutput=False)
    b2_ext = nc.declare_dram_parameter("b2", [W], F32, isOutput=False)
    out_ext = nc.declare_dram_parameter("out", [128, NCH // 2, 2, 2, 512],
                                        F16, isOutput=True)
    with tile.TileContext(nc) as tc:
        if repeat == 1:
            _emit(tc, r_ext, q_ext, w1_ext, b1_ext, w2_ext, b2_ext, out_ext)
        else:
            with tc.For_i(0, repeat, 1, staggered_reset=True):
                _emit(tc, r_ext, q_ext, w1_ext, b1_ext, w2_ext, b2_ext,
                      out_ext)
    nc.compile()
    return nc


def pack_weights(Q):
    """qstack tile 0 = y0 rows (w 0:96); tiles 1..6 rows follow SEGS3 (and
    its +3/+4 mirror): row p of tile t = Q[(y, w0 + p - a)].  Laid out
    [128, KT, IJ] fp16; tile 0 rows 96:128 zero."""
    Q = np.asarray(Q, np.float32)
    qmat = Q.transpose(2, 3, 0, 1).reshape(NY * W, IJ)      # [(y,w), (i,j)]
    qstack = np.zeros((KT, 128, IJ), np.float16)
    qstack[0, 0:W] = qmat[0:W].astype(np.float16)
    for (t, a, b, y, w0) in SEGS3:
        for u in range(2):
            rows = qmat[96 * (y + 4 * u) + w0:96 * (y + 4 * u) + w0 + b - a]
            qstack[t + 3 * u, a:b] = rows.astype(np.float16)
    return np.ascontiguousarray(qstack.transpose(1, 0, 2))


def make_in_map(inputs, core):
    r = np.ascontiguousarray(np.asarray(inputs["r"], np.float32))
    return dict(
        r=r[core * NP:(core + 1) * NP],
        qstack=pack_weights(inputs["Q"]),
        w1=np.ascontiguousarray(
            np.asarray(inputs["W1"], np.float32).astype(np.float16)),
        b1=np.ascontiguousarray(np.asarray(inputs["b1"], np.float32)),
        w2=np.ascontiguousarray(
            np.asarray(inputs["W2"], np.float32).astype(np.float16)),
        b2=np.ascontiguousarray(np.asarray(inputs["b2"], np.float32)),
    )


# device element a[i, cp, cpar, h, 128*j + p] = out[z, 128*h + i] with
# z = 64*p + 4*(2*cp + cpar) + j
_C = np.arange(NP)  # flat (cp, cpar, j, p) index = 1024*cp + 512*cpar + c
_ZMAP = (64 * (_C % 128) + 4 * (_C // 512) + (_C // 128) % 4)


def unscramble(a):
    """Device out [128, NCH//2, 2, 2, 512] -> [NP, IJ]."""
    a = np.asarray(a).reshape(128, NCH // 2, 2, 2, 512)
    # -> [h, i, cp, cpar, z2] -> [IJ, NP]
    a2 = a.transpose(3, 0, 1, 2, 4).reshape(IJ, NP)
    out = np.empty((NP, IJ), dtype=a2.dtype)
    out[_ZMAP] = a2.T
    return out


_NC_CACHE = None


def _get_nc():
    global _NC_CACHE
    if _NC_CACHE is None:
        _NC_CACHE = build_nc()
    return _NC_CACHE


def kernel(r, Q, W1, b1, W2, b2, K0):
    r = np.ascontiguousarray(np.asarray(r, dtype=np.float32))
    inputs = dict(r=r, Q=Q, W1=W1, b1=b1, W2=W2, b2=b2)
    in_maps = [make_in_map(inputs, i) for i in range(N_CORES)]
    res = run_bass_kernel_spmd(_get_nc(), in_maps, list(range(N_CORES)))
    out = np.concatenate([unscramble(res.results[i]["out"])
                          for i in range(N_CORES)], 0)
    out = out.reshape(N_TOTAL, 16, 16).astype(np.float32)
    # exact reference semantics for |r| == 0 points (K0 fallback)
    zero = ~(np.linalg.norm(r, axis=1) > 0.0)
    if zero.any():
        out[zero] = np.asarray(K0, np.float32)[None]
    return out
